# revision 1
# baseline (speedup 1.0000x reference)
"""AxialAttention (width=False) with the dominant qkv 1x1-conv matmul executed
data-parallel across 8 TRN2 NeuronCores (fp32r tensor-engine matmuls), and the
remaining attention arithmetic on host.

Sharding: batch N=16 -> 2 images per core. Each core computes
qkv[o, (b,h)] = w_qkv @ x_b for its shard (6.6 GFLOP/core of the 62.7 GFLOP
total; the qkv projection is 84% of all FLOPs in this module).
"""
import sys, os

sys.path.insert(0, "/opt/trn_rl_repo")
_DIR = os.path.dirname(os.path.abspath(__file__))
if _DIR not in sys.path:
    sys.path.insert(0, _DIR)

import numpy as np

IN_PLANES = 512
OUT_PLANES = 512
GROUPS = 8
K = 56
GP = OUT_PLANES // GROUPS
N = 16
EPS = 1e-5
NCORE = 8
P = 128
F = (N // NCORE) * K * K          # per-core (b,h) columns = 6272
O2 = 2 * OUT_PLANES               # 1024

_CACHE = {}


def _split_waits(nc, mybir, limit=1):
    ctr = 0
    for bb in nc.main_func.blocks:
        insts = list(bb.instructions)
        newlist = []
        changed = False
        for ins in insts:
            si = ins.sync_info
            ow = list(si.on_wait) if si is not None and si.on_wait else []
            if len(ow) > limit:
                changed = True
                excess, keep = ow[:-limit], ow[-limit:]
                for i in range(0, len(excess), limit):
                    ctr += 1
                    nop = mybir.InstNoOp(name=f"WSPLIT-{ctr}", ins=[], outs=[])
                    nop.engine = ins.engine
                    nop.sync_info = mybir.SyncInfo(on_wait=list(excess[i:i + limit]),
                                                   on_update=[])
                    nc.register_instruction(nop, overwrite=True)
                    newlist.append(nop)
                ins.sync_info = mybir.SyncInfo(
                    on_wait=list(keep),
                    on_update=list(si.on_update) if si.on_update else [])
            newlist.append(ins)
        if changed:
            bb.instructions = newlist
    return ctr


def _build():
    import concourse.bass as bass
    import concourse.mybir as mybir
    import concourse.tile as tile
    F32 = mybir.dt.float32
    F32R = mybir.dt.float32r
    AF = mybir.ActivationFunctionType

    nc = bass.Bass("TRN2", target_bir_lowering=False, debug=False, num_devices=NCORE)
    X_d = nc.declare_dram_parameter("xin", [IN_PLANES, F], F32, isOutput=False)
    W_d = nc.declare_dram_parameter("wqkv", [IN_PLANES, O2], F32, isOutput=False)
    Y_d = nc.declare_dram_parameter("qkv", [O2, F], F32, isOutput=True)

    NCH = 14
    FCH = F // NCH  # 448

    with tile.TileContext(nc, num_cores=NCORE) as tc:
        with (
            tc.tile_pool(name="const", bufs=1) as const,
            tc.tile_pool(name="xin", bufs=4) as xin,
            tc.tile_pool(name="outp", bufs=4) as outp,
            tc.tile_pool(name="ps", bufs=4, space="PSUM") as ps,
        ):
            w = const.tile([P, 4, O2], F32R)
            wf = xin.tile([P, 4, O2], F32, tag="wld")
            nc.sync.dma_start(wf[:], W_d.ap().rearrange("(ko p) o -> p ko o", p=P))
            nc.vector.tensor_copy(w[:], wf[:])
            for ch in range(NCH):
                xf = xin.tile([P, 4, FCH], F32, tag="xf")
                nc.sync.dma_start(
                    xf[:],
                    X_d.ap().rearrange("(ko p) f -> p ko f", p=P)[:, :, ch * FCH:(ch + 1) * FCH])
                xr = xin.tile([P, 4, FCH], F32R, tag="xr")
                nc.vector.tensor_copy(xr[:], xf[:])
                for m in range(O2 // P):
                    pt = ps.tile([P, FCH], F32, tag="qkvp")
                    for k in range(4):
                        nc.tensor.matmul(pt[:], w[:, k, m * P:(m + 1) * P], xr[:, k],
                                         start=(k == 0), stop=(k == 3))
                    ot = outp.tile([P, FCH], F32, tag="ot")
                    nc.scalar.activation(ot[:], pt[:], AF.Copy)
                    nc.sync.dma_start(
                        Y_d.ap()[m * P:(m + 1) * P, ch * FCH:(ch + 1) * FCH], ot[:])
    _split_waits(nc, mybir, 1)
    return nc


def _get_nc():
    if "nc" not in _CACHE:
        _CACHE["nc"] = _build()
    return _CACHE["nc"]


def _run_device_qkv(x):
    """x: [N, C, K, K] f32 -> qkv [N*K(w), O2, K(h)] f32 via 8-core SPMD."""
    from concourse import bass_utils
    nc = _get_nc()
    npc = N // NCORE
    in_maps = []
    for c in range(NCORE):
        xs = x[c * npc:(c + 1) * npc]                    # [2, C, H, W]
        xt = np.ascontiguousarray(xs.transpose(1, 0, 3, 2).reshape(IN_PLANES, F))
        in_maps.append({"xin": xt, "wqkv": _CACHE["wT"]})
    res = bass_utils.run_bass_kernel_spmd(nc, in_maps, core_ids=list(range(NCORE)))
    _CACHE["last_exec_ns"] = res.exec_time_ns
    out = np.empty((N * K, O2, K), np.float32)
    for c in range(NCORE):
        q = res.results[c]["qkv"]                        # [O2, (b,h)] b=(n_l,w)
        out[c * npc * K:(c + 1) * npc * K] = (
            q.reshape(O2, npc * K, K).transpose(1, 0, 2))
    return out


def kernel(x, w_qkv, relative, g_qkv, b_qkv, g_sim, b_sim, g_out, b_out):
    x = np.asarray(x, np.float32)
    w_qkv = np.asarray(w_qkv, np.float32)
    relative = np.asarray(relative, np.float32)
    g_qkv = np.asarray(g_qkv, np.float32); b_qkv = np.asarray(b_qkv, np.float32)
    g_sim = np.asarray(g_sim, np.float32); b_sim = np.asarray(b_sim, np.float32)
    g_out = np.asarray(g_out, np.float32); b_out = np.asarray(b_out, np.float32)

    _CACHE["wT"] = np.ascontiguousarray(w_qkv.T)         # [C, O2]

    # ---- device: qkv projection (84% of FLOPs), data-parallel over N ----
    qkv = _run_device_qkv(x)                             # [b=N*W, O2, H]

    # ---- host: BN + axial attention (matches reference bit-for-bit in fp32) ----
    b = qkv.shape[0]
    mean = qkv.mean(axis=(0, 2), keepdims=True)
    var = qkv.var(axis=(0, 2), keepdims=True)
    qkvn = (qkv - mean) / np.sqrt(var + EPS) * g_qkv.reshape(1, -1, 1) + b_qkv.reshape(1, -1, 1)
    qkvn = qkvn.reshape(b, GROUPS, 2 * GP, K)
    q = qkvn[:, :, :GP // 2]
    k = qkvn[:, :, GP // 2:GP]
    v = qkvn[:, :, GP:]

    qi = np.arange(K)[None, :]
    ki = np.arange(K)[:, None]
    flat_idx = (ki - qi + K - 1).reshape(-1)
    all_emb = relative[:, flat_idx].reshape(2 * GP, K, K)
    q_emb = all_emb[:GP // 2]
    k_emb = all_emb[GP // 2:GP]
    v_emb = all_emb[GP:]

    qr = np.einsum("bgci,cij->bgij", q, q_emb, optimize=True)
    kr = np.einsum("bgci,cij->bgij", k, k_emb, optimize=True).transpose(0, 1, 3, 2)
    qk = np.einsum("bgci,bgcj->bgij", q, k, optimize=True)
    stacked = np.concatenate([qk, qr, kr], axis=1)
    sm = stacked.mean(axis=(0, 2, 3), keepdims=True)
    sv_ = stacked.var(axis=(0, 2, 3), keepdims=True)
    stacked = (stacked - sm) / np.sqrt(sv_ + EPS) * g_sim.reshape(1, -1, 1, 1) + b_sim.reshape(1, -1, 1, 1)
    sim = stacked.reshape(b, 3, GROUPS, K, K).sum(axis=1)
    sim = sim - sim.max(axis=3, keepdims=True)
    np.exp(sim, out=sim)
    sim /= sim.sum(axis=3, keepdims=True)
    svv = np.einsum("bgij,bgcj->bgci", sim, v, optimize=True)
    sve = np.einsum("bgij,cij->bgci", sim, v_emb, optimize=True)
    out = np.concatenate([svv, sve], axis=-1).reshape(b, 2 * OUT_PLANES, K)
    om = out.mean(axis=(0, 2), keepdims=True)
    ov = out.var(axis=(0, 2), keepdims=True)
    out = (out - om) / np.sqrt(ov + EPS) * g_out.reshape(1, -1, 1) + b_out.reshape(1, -1, 1)
    out = out.reshape(N, K, OUT_PLANES, 2, K).sum(axis=3)
    return np.ascontiguousarray(out.transpose(0, 2, 3, 1)).astype(np.float32)



# revision 2
# speedup vs baseline: 1.3967x; 1.3967x over previous
"""AxialAttention (width=False) with the dominant qkv 1x1-conv matmul executed
data-parallel across 8 TRN2 NeuronCores, and the remaining attention
arithmetic on host.

Sharding: batch N=16 -> 2 images per core. Each core computes
qkv[o, (b,h)] = w_qkv @ x_b for its shard (6.6 GFLOP/core of the 62.7 GFLOP
total; the qkv projection is 84% of all FLOPs in this module).

Device kernel v2: fp16 inputs + fp16 output (halves HBM traffic vs fp32 —
the v1 kernel was DMA-bound at 85% DMA occupancy), PE does fp16 matmuls at
1 cycle/row (same rate as fp32r), PSUM drains split across Scalar+Vector
engines, one batched output DMA per 448-column chunk.
"""
import sys, os

sys.path.insert(0, "/opt/trn_rl_repo")
_DIR = os.path.dirname(os.path.abspath(__file__))
if _DIR not in sys.path:
    sys.path.insert(0, _DIR)

import numpy as np

IN_PLANES = 512
OUT_PLANES = 512
GROUPS = 8
K = 56
GP = OUT_PLANES // GROUPS
N = 16
EPS = 1e-5
NCORE = 8
P = 128
F = (N // NCORE) * K * K          # per-core (b,h) columns = 6272
O2 = 2 * OUT_PLANES               # 1024

_CACHE = {}


def _split_waits(nc, mybir, limit=1):
    ctr = 0
    for bb in nc.main_func.blocks:
        insts = list(bb.instructions)
        newlist = []
        changed = False
        for ins in insts:
            si = ins.sync_info
            ow = list(si.on_wait) if si is not None and si.on_wait else []
            if len(ow) > limit:
                changed = True
                excess, keep = ow[:-limit], ow[-limit:]
                for i in range(0, len(excess), limit):
                    ctr += 1
                    nop = mybir.InstNoOp(name=f"WSPLIT-{ctr}", ins=[], outs=[])
                    nop.engine = ins.engine
                    nop.sync_info = mybir.SyncInfo(on_wait=list(excess[i:i + limit]),
                                                   on_update=[])
                    nc.register_instruction(nop, overwrite=True)
                    newlist.append(nop)
                ins.sync_info = mybir.SyncInfo(
                    on_wait=list(keep),
                    on_update=list(si.on_update) if si.on_update else [])
            newlist.append(ins)
        if changed:
            bb.instructions = newlist
    return ctr


def _build():
    import concourse.bass as bass
    import concourse.mybir as mybir
    import concourse.tile as tile
    F32 = mybir.dt.float32
    F16 = mybir.dt.float16
    AF = mybir.ActivationFunctionType

    nc = bass.Bass("TRN2", target_bir_lowering=False, debug=False, num_devices=NCORE)
    X_d = nc.declare_dram_parameter("xin", [IN_PLANES, F], F16, isOutput=False)
    W_d = nc.declare_dram_parameter("wqkv", [IN_PLANES, O2], F16, isOutput=False)
    Y_d = nc.declare_dram_parameter("qkv", [O2, F], F16, isOutput=True)

    NCH = 14
    FCH = F // NCH  # 448

    with tile.TileContext(nc, num_cores=NCORE) as tc:
        with (
            tc.tile_pool(name="const", bufs=1) as const,
            tc.tile_pool(name="xin", bufs=4) as xin,
            tc.tile_pool(name="outp", bufs=3) as outp,
            tc.tile_pool(name="ps", bufs=8, space="PSUM") as ps,
        ):
            w = const.tile([P, 4, O2], F16)
            nc.sync.dma_start(w[:], W_d.ap().rearrange("(ko p) o -> p ko o", p=P))
            for ch in range(NCH):
                xf = xin.tile([P, 4, FCH], F16, tag="xf")
                nc.sync.dma_start(
                    xf[:],
                    X_d.ap().rearrange("(ko p) f -> p ko f", p=P)[:, :, ch * FCH:(ch + 1) * FCH])
                st = outp.tile([P, O2 // P, FCH], F16, tag="st")
                for m in range(O2 // P):
                    pt = ps.tile([P, FCH], F32, tag="qkvp")
                    for k in range(4):
                        nc.tensor.matmul(pt[:], w[:, k, m * P:(m + 1) * P], xf[:, k],
                                         start=(k == 0), stop=(k == 3))
                    if m % 2 == 0:
                        nc.scalar.activation(st[:, m], pt[:], AF.Copy)
                    else:
                        nc.vector.tensor_copy(st[:, m], pt[:])
                nc.sync.dma_start(
                    Y_d.ap().rearrange("(m p) f -> p m f", p=P)[:, :, ch * FCH:(ch + 1) * FCH],
                    st[:])
    _split_waits(nc, mybir, 1)
    return nc


def _get_nc():
    if "nc" not in _CACHE:
        _CACHE["nc"] = _build()
    return _CACHE["nc"]


def _make_in_maps(x):
    npc = N // NCORE
    in_maps = []
    for c in range(NCORE):
        xs = x[c * npc:(c + 1) * npc]                    # [2, C, H, W]
        xt = np.ascontiguousarray(
            xs.transpose(1, 0, 3, 2).reshape(IN_PLANES, F).astype(np.float16))
        in_maps.append({"xin": xt, "wqkv": _CACHE["wT"]})
    return in_maps


def _run_device_qkv(x):
    """x: [N, C, K, K] f32 -> qkv [N*K(w), O2, K(h)] f32 via 8-core SPMD."""
    from concourse import bass_utils
    nc = _get_nc()
    npc = N // NCORE
    in_maps = _make_in_maps(x)
    res = bass_utils.run_bass_kernel_spmd(nc, in_maps, core_ids=list(range(NCORE)))
    _CACHE["last_exec_ns"] = res.exec_time_ns
    out = np.empty((N * K, O2, K), np.float32)
    for c in range(NCORE):
        q = np.asarray(res.results[c]["qkv"], np.float32)   # [O2, (b,h)] b=(n_l,w)
        out[c * npc * K:(c + 1) * npc * K] = (
            q.reshape(O2, npc * K, K).transpose(1, 0, 2))
    return out


def kernel(x, w_qkv, relative, g_qkv, b_qkv, g_sim, b_sim, g_out, b_out):
    x = np.asarray(x, np.float32)
    w_qkv = np.asarray(w_qkv, np.float32)
    relative = np.asarray(relative, np.float32)
    g_qkv = np.asarray(g_qkv, np.float32); b_qkv = np.asarray(b_qkv, np.float32)
    g_sim = np.asarray(g_sim, np.float32); b_sim = np.asarray(b_sim, np.float32)
    g_out = np.asarray(g_out, np.float32); b_out = np.asarray(b_out, np.float32)

    _CACHE["wT"] = np.ascontiguousarray(w_qkv.T.astype(np.float16))  # [C, O2]

    # ---- device: qkv projection (84% of FLOPs), data-parallel over N ----
    qkv = _run_device_qkv(x)                             # [b=N*W, O2, H]

    # ---- host: BN + axial attention (matches reference in fp32) ----
    b = qkv.shape[0]
    mean = qkv.mean(axis=(0, 2), keepdims=True)
    var = qkv.var(axis=(0, 2), keepdims=True)
    qkvn = (qkv - mean) / np.sqrt(var + EPS) * g_qkv.reshape(1, -1, 1) + b_qkv.reshape(1, -1, 1)
    qkvn = qkvn.reshape(b, GROUPS, 2 * GP, K)
    q = qkvn[:, :, :GP // 2]
    k = qkvn[:, :, GP // 2:GP]
    v = qkvn[:, :, GP:]

    qi = np.arange(K)[None, :]
    ki = np.arange(K)[:, None]
    flat_idx = (ki - qi + K - 1).reshape(-1)
    all_emb = relative[:, flat_idx].reshape(2 * GP, K, K)
    q_emb = all_emb[:GP // 2]
    k_emb = all_emb[GP // 2:GP]
    v_emb = all_emb[GP:]

    qr = np.einsum("bgci,cij->bgij", q, q_emb, optimize=True)
    kr = np.einsum("bgci,cij->bgij", k, k_emb, optimize=True).transpose(0, 1, 3, 2)
    qk = np.einsum("bgci,bgcj->bgij", q, k, optimize=True)
    stacked = np.concatenate([qk, qr, kr], axis=1)
    sm = stacked.mean(axis=(0, 2, 3), keepdims=True)
    sv_ = stacked.var(axis=(0, 2, 3), keepdims=True)
    stacked = (stacked - sm) / np.sqrt(sv_ + EPS) * g_sim.reshape(1, -1, 1, 1) + b_sim.reshape(1, -1, 1, 1)
    sim = stacked.reshape(b, 3, GROUPS, K, K).sum(axis=1)
    sim = sim - sim.max(axis=3, keepdims=True)
    np.exp(sim, out=sim)
    sim /= sim.sum(axis=3, keepdims=True)
    svv = np.einsum("bgij,bgcj->bgci", sim, v, optimize=True)
    sve = np.einsum("bgij,cij->bgci", sim, v_emb, optimize=True)
    out = np.concatenate([svv, sve], axis=-1).reshape(b, 2 * OUT_PLANES, K)
    om = out.mean(axis=(0, 2), keepdims=True)
    ov = out.var(axis=(0, 2), keepdims=True)
    out = (out - om) / np.sqrt(ov + EPS) * g_out.reshape(1, -1, 1) + b_out.reshape(1, -1, 1)
    out = out.reshape(N, K, OUT_PLANES, 2, K).sum(axis=3)
    return np.ascontiguousarray(out.transpose(0, 2, 3, 1)).astype(np.float32)


# revision 4
# speedup vs baseline: 1.4024x; 1.0041x over previous
"""AxialAttention (width=False) with the dominant qkv 1x1-conv matmul executed
data-parallel across 8 TRN2 NeuronCores, and the remaining attention
arithmetic on host.

Sharding: batch N=16 -> 2 images per core. Each core computes
qkv[o, (b,h)] = w_qkv @ x_b for its shard (6.6 GFLOP/core of the 62.7 GFLOP
total; the qkv projection is 84% of all FLOPs in this module).

Device kernel v3: fp16 in/out (halves HBM traffic; fp16 matmul runs at the
same 1 cycle/row as fp32r). k-outer accumulation across all 8 PSUM banks so
the first matmuls only depend on the ko=0 slices of w and x; w and x are
DMAed per-ko so the PE starts ~6us earlier. Outputs drain via alternating
Scalar/Vector copies and leave on the Scalar HWDGE ring while inputs use the
Sync ring.
"""
import sys, os

sys.path.insert(0, "/opt/trn_rl_repo")
_DIR = os.path.dirname(os.path.abspath(__file__))
if _DIR not in sys.path:
    sys.path.insert(0, _DIR)

import numpy as np

IN_PLANES = 512
OUT_PLANES = 512
GROUPS = 8
K = 56
GP = OUT_PLANES // GROUPS
N = 16
EPS = 1e-5
NCORE = 8
P = 128
F = (N // NCORE) * K * K          # per-core (b,h) columns = 6272
O2 = 2 * OUT_PLANES               # 1024
KO = IN_PLANES // P               # 4 contraction passes
MT = O2 // P                      # 8 output tiles
NCH = 14
FCH = F // NCH                    # 448

_CACHE = {}


def _split_waits(nc, mybir, limit=1):
    ctr = 0
    for bb in nc.main_func.blocks:
        insts = list(bb.instructions)
        newlist = []
        changed = False
        for ins in insts:
            si = ins.sync_info
            ow = list(si.on_wait) if si is not None and si.on_wait else []
            if len(ow) > limit:
                changed = True
                excess, keep = ow[:-limit], ow[-limit:]
                for i in range(0, len(excess), limit):
                    ctr += 1
                    nop = mybir.InstNoOp(name=f"WSPLIT-{ctr}", ins=[], outs=[])
                    nop.engine = ins.engine
                    nop.sync_info = mybir.SyncInfo(on_wait=list(excess[i:i + limit]),
                                                   on_update=[])
                    nc.register_instruction(nop, overwrite=True)
                    newlist.append(nop)
                ins.sync_info = mybir.SyncInfo(
                    on_wait=list(keep),
                    on_update=list(si.on_update) if si.on_update else [])
            newlist.append(ins)
        if changed:
            bb.instructions = newlist
    return ctr


def _build():
    import concourse.bass as bass
    import concourse.mybir as mybir
    import concourse.tile as tile
    F32 = mybir.dt.float32
    F16 = mybir.dt.float16
    AF = mybir.ActivationFunctionType

    nc = bass.Bass("TRN2", target_bir_lowering=False, debug=False, num_devices=NCORE)
    X_d = nc.declare_dram_parameter("xin", [IN_PLANES, F], F16, isOutput=False)
    W_d = nc.declare_dram_parameter("wqkv", [IN_PLANES, O2], F16, isOutput=False)
    Y_d = nc.declare_dram_parameter("qkv", [O2, F], F16, isOutput=True)

    with tile.TileContext(nc, num_cores=NCORE) as tc:
        with (
            tc.tile_pool(name="wp", bufs=1) as wp,
            tc.tile_pool(name="xin", bufs=12) as xin,
            tc.tile_pool(name="outp", bufs=3) as outp,
            tc.tile_pool(name="ps", bufs=8, space="PSUM") as ps,
        ):
            Wr = W_d.ap().rearrange("(ko p) o -> p ko o", p=P)
            Xr = X_d.ap().rearrange("(ko p) f -> p ko f", p=P)
            Yr = Y_d.ap().rearrange("(m p) f -> p m f", p=P)

            w = wp.tile([P, KO, O2], F16)

            def load_x(ch):
                xt = []
                for ko in range(KO):
                    t = xin.tile([P, FCH], F16, tag="xf")
                    nc.sync.dma_start(
                        t[:], Xr[:, ko, ch * FCH:(ch + 1) * FCH])
                    xt.append(t)
                return xt

            # dispatch order on the Sync ring: w-ko0 first (gates the very
            # first matmul), then chunk0's x, then the rest of w.
            nc.sync.dma_start(w[:, 0], Wr[:, 0])
            x_pending = load_x(0)
            for ko in range(1, KO):
                nc.sync.dma_start(w[:, ko], Wr[:, ko])

            for ch in range(NCH):
                xt = x_pending
                if ch + 1 < NCH:
                    x_pending = load_x(ch + 1)
                pts = [ps.tile([P, FCH], F32, tag="qkvp", name=f"pt{ch}_{m}")
                       for m in range(MT)]
                for k in range(KO):
                    for m in range(MT):
                        nc.tensor.matmul(pts[m][:], w[:, k, m * P:(m + 1) * P],
                                         xt[k][:],
                                         start=(k == 0), stop=(k == KO - 1))
                st = outp.tile([P, MT, FCH], F16, tag="st")
                for m in range(MT):
                    if m % 2 == 0:
                        nc.scalar.activation(st[:, m], pts[m][:], AF.Copy)
                    else:
                        nc.vector.tensor_copy(st[:, m], pts[m][:])
                    if ch == NCH - 1:
                        # last chunk: ship each m-tile as soon as it drains
                        nc.scalar.dma_start(
                            Yr[:, m, ch * FCH:(ch + 1) * FCH], st[:, m])
                if ch < NCH - 1:
                    nc.scalar.dma_start(
                        Yr[:, :, ch * FCH:(ch + 1) * FCH], st[:])
    _split_waits(nc, mybir, 1)
    return nc


def _get_nc():
    if "nc" not in _CACHE:
        _CACHE["nc"] = _build()
    return _CACHE["nc"]


def _make_in_maps(x):
    npc = N // NCORE
    in_maps = []
    for c in range(NCORE):
        xs = x[c * npc:(c + 1) * npc]                    # [2, C, H, W]
        xt = np.ascontiguousarray(
            xs.transpose(1, 0, 3, 2).reshape(IN_PLANES, F).astype(np.float16))
        in_maps.append({"xin": xt, "wqkv": _CACHE["wT"]})
    return in_maps


def _run_device_qkv(x):
    """x: [N, C, K, K] f32 -> qkv [N*K(w), O2, K(h)] f32 via 8-core SPMD."""
    from concourse import bass_utils
    nc = _get_nc()
    npc = N // NCORE
    in_maps = _make_in_maps(x)
    res = bass_utils.run_bass_kernel_spmd(nc, in_maps, core_ids=list(range(NCORE)))
    _CACHE["last_exec_ns"] = res.exec_time_ns
    out = np.empty((N * K, O2, K), np.float32)
    for c in range(NCORE):
        q = np.asarray(res.results[c]["qkv"], np.float32)   # [O2, (b,h)] b=(n_l,w)
        out[c * npc * K:(c + 1) * npc * K] = (
            q.reshape(O2, npc * K, K).transpose(1, 0, 2))
    return out


def kernel(x, w_qkv, relative, g_qkv, b_qkv, g_sim, b_sim, g_out, b_out):
    x = np.asarray(x, np.float32)
    w_qkv = np.asarray(w_qkv, np.float32)
    relative = np.asarray(relative, np.float32)
    g_qkv = np.asarray(g_qkv, np.float32); b_qkv = np.asarray(b_qkv, np.float32)
    g_sim = np.asarray(g_sim, np.float32); b_sim = np.asarray(b_sim, np.float32)
    g_out = np.asarray(g_out, np.float32); b_out = np.asarray(b_out, np.float32)

    _CACHE["wT"] = np.ascontiguousarray(w_qkv.T.astype(np.float16))  # [C, O2]

    # ---- device: qkv projection (84% of FLOPs), data-parallel over N ----
    qkv = _run_device_qkv(x)                             # [b=N*W, O2, H]

    # ---- host: BN + axial attention (matches reference in fp32) ----
    b = qkv.shape[0]
    mean = qkv.mean(axis=(0, 2), keepdims=True)
    var = qkv.var(axis=(0, 2), keepdims=True)
    qkvn = (qkv - mean) / np.sqrt(var + EPS) * g_qkv.reshape(1, -1, 1) + b_qkv.reshape(1, -1, 1)
    qkvn = qkvn.reshape(b, GROUPS, 2 * GP, K)
    q = qkvn[:, :, :GP // 2]
    k = qkvn[:, :, GP // 2:GP]
    v = qkvn[:, :, GP:]

    qi = np.arange(K)[None, :]
    ki = np.arange(K)[:, None]
    flat_idx = (ki - qi + K - 1).reshape(-1)
    all_emb = relative[:, flat_idx].reshape(2 * GP, K, K)
    q_emb = all_emb[:GP // 2]
    k_emb = all_emb[GP // 2:GP]
    v_emb = all_emb[GP:]

    qr = np.einsum("bgci,cij->bgij", q, q_emb, optimize=True)
    kr = np.einsum("bgci,cij->bgij", k, k_emb, optimize=True).transpose(0, 1, 3, 2)
    qk = np.einsum("bgci,bgcj->bgij", q, k, optimize=True)
    stacked = np.concatenate([qk, qr, kr], axis=1)
    sm = stacked.mean(axis=(0, 2, 3), keepdims=True)
    sv_ = stacked.var(axis=(0, 2, 3), keepdims=True)
    stacked = (stacked - sm) / np.sqrt(sv_ + EPS) * g_sim.reshape(1, -1, 1, 1) + b_sim.reshape(1, -1, 1, 1)
    sim = stacked.reshape(b, 3, GROUPS, K, K).sum(axis=1)
    sim = sim - sim.max(axis=3, keepdims=True)
    np.exp(sim, out=sim)
    sim /= sim.sum(axis=3, keepdims=True)
    svv = np.einsum("bgij,bgcj->bgci", sim, v, optimize=True)
    sve = np.einsum("bgij,cij->bgci", sim, v_emb, optimize=True)
    out = np.concatenate([svv, sve], axis=-1).reshape(b, 2 * OUT_PLANES, K)
    om = out.mean(axis=(0, 2), keepdims=True)
    ov = out.var(axis=(0, 2), keepdims=True)
    out = (out - om) / np.sqrt(ov + EPS) * g_out.reshape(1, -1, 1) + b_out.reshape(1, -1, 1)
    out = out.reshape(N, K, OUT_PLANES, 2, K).sum(axis=3)
    return np.ascontiguousarray(out.transpose(0, 2, 3, 1)).astype(np.float32)


# revision 8
# speedup vs baseline: 1.4576x; 1.0394x over previous
"""AxialAttention (width=False) with the dominant qkv 1x1-conv matmul executed
data-parallel across 8 TRN2 NeuronCores, and the remaining attention
arithmetic on host.

Sharding: batch N=16 -> 2 images per core. Each core computes
qkv[o, (b,h)] = w_qkv @ x_b for its shard (6.6 GFLOP/core of the 62.7 GFLOP
total; the qkv projection is 84% of all FLOPs in this module).

Device kernel v4: fp16 in/out (halves HBM traffic; fp16 matmul runs at the
same 1 cycle/row as fp32r). k-outer accumulation across all 8 PSUM banks;
w is held as four per-ko tiles so the first matmul only waits on the first
256KB DMA; chunk 0's x rides the Scalar HWDGE ring in parallel with w on the
Sync ring. Dummy matmuls on memset tiles pre-ramp the PE HAM clock gate
while the first DMAs are in flight. The final chunk is split in half to
shorten the drain tail. Outputs leave on the Scalar ring.
"""
import sys, os

sys.path.insert(0, "/opt/trn_rl_repo")
_DIR = os.path.dirname(os.path.abspath(__file__))
if _DIR not in sys.path:
    sys.path.insert(0, _DIR)

import numpy as np

IN_PLANES = 512
OUT_PLANES = 512
GROUPS = 8
K = 56
GP = OUT_PLANES // GROUPS
N = 16
EPS = 1e-5
NCORE = 8
P = 128
F = (N // NCORE) * K * K          # per-core (b,h) columns = 6272
O2 = 2 * OUT_PLANES               # 1024
KO = IN_PLANES // P               # 4 contraction passes
MT = O2 // P                      # 8 output tiles
FCH = 448
CHUNKS = [(i * FCH, FCH) for i in range(13)] + [(13 * FCH, 224), (13 * FCH + 224, 224)]
N_WARM = 12                       # dummy matmuls to ramp the HAM clock gate

_CACHE = {}


def _split_waits(nc, mybir, limit=1):
    ctr = 0
    for bb in nc.main_func.blocks:
        insts = list(bb.instructions)
        newlist = []
        changed = False
        for ins in insts:
            si = ins.sync_info
            ow = list(si.on_wait) if si is not None and si.on_wait else []
            if len(ow) > limit:
                changed = True
                excess, keep = ow[:-limit], ow[-limit:]
                for i in range(0, len(excess), limit):
                    ctr += 1
                    nop = mybir.InstNoOp(name=f"WSPLIT-{ctr}", ins=[], outs=[])
                    nop.engine = ins.engine
                    nop.sync_info = mybir.SyncInfo(on_wait=list(excess[i:i + limit]),
                                                   on_update=[])
                    nc.register_instruction(nop, overwrite=True)
                    newlist.append(nop)
                ins.sync_info = mybir.SyncInfo(
                    on_wait=list(keep),
                    on_update=list(si.on_update) if si.on_update else [])
            newlist.append(ins)
        if changed:
            bb.instructions = newlist
    return ctr


def _build():
    import concourse.bass as bass
    import concourse.mybir as mybir
    import concourse.tile as tile
    F32 = mybir.dt.float32
    F16 = mybir.dt.float16
    AF = mybir.ActivationFunctionType

    nc = bass.Bass("TRN2", target_bir_lowering=False, debug=False, num_devices=NCORE)
    X_d = nc.declare_dram_parameter("xin", [IN_PLANES, F], F16, isOutput=False)
    W_d = nc.declare_dram_parameter("wqkv", [IN_PLANES, O2], F16, isOutput=False)
    Y_d = nc.declare_dram_parameter("qkv", [O2, F], F16, isOutput=True)

    with tile.TileContext(nc, num_cores=NCORE) as tc:
        with (
            tc.tile_pool(name="wp", bufs=1) as wp,
            tc.tile_pool(name="xin", bufs=3) as xin,
            tc.tile_pool(name="outp", bufs=3) as outp,
            tc.tile_pool(name="ps", bufs=8, space="PSUM") as ps,
        ):
            Wr = W_d.ap().rearrange("(ko p) o -> p ko o", p=P)
            Xr = X_d.ap().rearrange("(ko p) f -> p ko f", p=P)
            Yr = Y_d.ap().rearrange("(m p) f -> p m f", p=P)

            # warmup inputs: dep-free garbage tiles
            wa = wp.tile([P, P], F16)
            wx = wp.tile([P, FCH], F16)
            nc.gpsimd.memset(wa[:], 0.0)
            nc.gpsimd.memset(wx[:], 0.0)
            for i in range(N_WARM):
                dpt = ps.tile([P, FCH], F32, tag="qkvp", name=f"warm{i}")
                nc.tensor.matmul(dpt[:], wa[:], wx[:], start=True, stop=True)

            # w as 4 per-ko tiles: the first matmul only waits on wt[0]'s DMA
            wt = []
            for ko in range(KO):
                t = wp.tile([P, O2], F16, name=f"w{ko}")
                nc.sync.dma_start(t[:], Wr[:, ko])
                wt.append(t)

            def load_x(ci, engine):
                off, fch = CHUNKS[ci]
                t = xin.tile([P, KO, FCH], F16, tag="xf", name=f"x{ci}")
                engine.dma_start(t[:, :, :fch], Xr[:, :, off:off + fch])
                return t

            # chunk 0 x on the Scalar ring, in parallel with w on the Sync ring
            x_pending = load_x(0, nc.scalar)

            for ci, (off, fch) in enumerate(CHUNKS):
                xt = x_pending
                if ci + 1 < len(CHUNKS):
                    x_pending = load_x(ci + 1, nc.sync)
                pts = [ps.tile([P, FCH], F32, tag="qkvp", name=f"pt{ci}_{m}")
                       for m in range(MT)]
                for k in range(KO):
                    for m in range(MT):
                        nc.tensor.matmul(pts[m][:, :fch],
                                         wt[k][:, m * P:(m + 1) * P],
                                         xt[:, k, :fch],
                                         start=(k == 0), stop=(k == KO - 1))
                st = outp.tile([P, MT, FCH], F16, tag="st", name=f"st{ci}")
                for m in range(MT):
                    if m % 2 == 0:
                        nc.scalar.activation(st[:, m, :fch], pts[m][:, :fch], AF.Copy)
                    else:
                        nc.vector.tensor_copy(st[:, m, :fch], pts[m][:, :fch])
                nc.scalar.dma_start(Yr[:, :, off:off + fch], st[:, :, :fch])
    _split_waits(nc, mybir, 1)
    return nc


def _get_nc():
    if "nc" not in _CACHE:
        _CACHE["nc"] = _build()
    return _CACHE["nc"]


def _make_in_maps(x):
    npc = N // NCORE
    in_maps = []
    for c in range(NCORE):
        xs = x[c * npc:(c + 1) * npc]                    # [2, C, H, W]
        xt = np.ascontiguousarray(
            xs.transpose(1, 0, 3, 2).reshape(IN_PLANES, F).astype(np.float16))
        in_maps.append({"xin": xt, "wqkv": _CACHE["wT"]})
    return in_maps


def _run_device_qkv(x):
    """x: [N, C, K, K] f32 -> qkv [N*K(w), O2, K(h)] f32 via 8-core SPMD."""
    from concourse import bass_utils
    nc = _get_nc()
    npc = N // NCORE
    in_maps = _make_in_maps(x)
    res = bass_utils.run_bass_kernel_spmd(nc, in_maps, core_ids=list(range(NCORE)))
    _CACHE["last_exec_ns"] = res.exec_time_ns
    out = np.empty((N * K, O2, K), np.float32)
    for c in range(NCORE):
        q = np.asarray(res.results[c]["qkv"], np.float32)   # [O2, (b,h)] b=(n_l,w)
        out[c * npc * K:(c + 1) * npc * K] = (
            q.reshape(O2, npc * K, K).transpose(1, 0, 2))
    return out


def kernel(x, w_qkv, relative, g_qkv, b_qkv, g_sim, b_sim, g_out, b_out):
    x = np.asarray(x, np.float32)
    w_qkv = np.asarray(w_qkv, np.float32)
    relative = np.asarray(relative, np.float32)
    g_qkv = np.asarray(g_qkv, np.float32); b_qkv = np.asarray(b_qkv, np.float32)
    g_sim = np.asarray(g_sim, np.float32); b_sim = np.asarray(b_sim, np.float32)
    g_out = np.asarray(g_out, np.float32); b_out = np.asarray(b_out, np.float32)

    _CACHE["wT"] = np.ascontiguousarray(w_qkv.T.astype(np.float16))  # [C, O2]

    # ---- device: qkv projection (84% of FLOPs), data-parallel over N ----
    qkv = _run_device_qkv(x)                             # [b=N*W, O2, H]

    # ---- host: BN + axial attention (matches reference in fp32) ----
    b = qkv.shape[0]
    mean = qkv.mean(axis=(0, 2), keepdims=True)
    var = qkv.var(axis=(0, 2), keepdims=True)
    qkvn = (qkv - mean) / np.sqrt(var + EPS) * g_qkv.reshape(1, -1, 1) + b_qkv.reshape(1, -1, 1)
    qkvn = qkvn.reshape(b, GROUPS, 2 * GP, K)
    q = qkvn[:, :, :GP // 2]
    k = qkvn[:, :, GP // 2:GP]
    v = qkvn[:, :, GP:]

    qi = np.arange(K)[None, :]
    ki = np.arange(K)[:, None]
    flat_idx = (ki - qi + K - 1).reshape(-1)
    all_emb = relative[:, flat_idx].reshape(2 * GP, K, K)
    q_emb = all_emb[:GP // 2]
    k_emb = all_emb[GP // 2:GP]
    v_emb = all_emb[GP:]

    qr = np.einsum("bgci,cij->bgij", q, q_emb, optimize=True)
    kr = np.einsum("bgci,cij->bgij", k, k_emb, optimize=True).transpose(0, 1, 3, 2)
    qk = np.einsum("bgci,bgcj->bgij", q, k, optimize=True)
    stacked = np.concatenate([qk, qr, kr], axis=1)
    sm = stacked.mean(axis=(0, 2, 3), keepdims=True)
    sv_ = stacked.var(axis=(0, 2, 3), keepdims=True)
    stacked = (stacked - sm) / np.sqrt(sv_ + EPS) * g_sim.reshape(1, -1, 1, 1) + b_sim.reshape(1, -1, 1, 1)
    sim = stacked.reshape(b, 3, GROUPS, K, K).sum(axis=1)
    sim = sim - sim.max(axis=3, keepdims=True)
    np.exp(sim, out=sim)
    sim /= sim.sum(axis=3, keepdims=True)
    svv = np.einsum("bgij,bgcj->bgci", sim, v, optimize=True)
    sve = np.einsum("bgij,cij->bgci", sim, v_emb, optimize=True)
    out = np.concatenate([svv, sve], axis=-1).reshape(b, 2 * OUT_PLANES, K)
    om = out.mean(axis=(0, 2), keepdims=True)
    ov = out.var(axis=(0, 2), keepdims=True)
    out = (out - om) / np.sqrt(ov + EPS) * g_out.reshape(1, -1, 1) + b_out.reshape(1, -1, 1)
    out = out.reshape(N, K, OUT_PLANES, 2, K).sum(axis=3)
    return np.ascontiguousarray(out.transpose(0, 2, 3, 1)).astype(np.float32)
